# revision 14
# baseline (speedup 1.0000x reference)
"""Trainium2 Bass kernel for nn_Discriminator_AddDim_ESSAAttn.

Network (per sample, C=128, 27x27 spatial, N=729 tokens):
  ESSA linear attention -> concat -> 1x1-conv FFN (+residual) ->
  3x3 conv/relu/pool x2 -> 3 FC layers -> [16] logits.
Batch 256 is sharded 32-per-core across 8 NeuronCores (pure data
parallel, weights replicated). Everything for one sample lives in SBUF;
weights are resident.

Layout strategy:
  - qkv computed token-major ([tokens, 384] psum tiles) so per-token
    stats (channel means/sums/norms) are per-partition scalars.
    Channel-mean subtraction is folded into the qkv weights host-side
    (centering is linear: w_q' = w_q - mean_j(w_q)).
  - q2-row-normalisation folded into per-partition tensor_scalar muls;
    k2 column (token-dim) L2 norm folded into a post-matmul per-channel
    scale using the gram matrix diag computed in the same PE pass.
  - kv/t2/attn/convs run channel-major; q2n is PE-transposed.
  - v (channel-major) and the ffn2 residual are accumulated directly in
    PSUM by extra matmuls (w_v / identity as stationary).
  - convs are 9 accumulating matmuls over shifted [C, H, W] views;
    conv2 batches 4 samples per matmul to keep the moving dim >= 256.
  - all matmuls use float32r (single-pass full-rate fp32 on the PE;
    plain fp32 is a 2-pass 4 cyc/col emulation).
"""
import sys

sys.path.insert(0, "/opt/trn_rl_repo")

import numpy as np

import concourse.bass as bass
import concourse.tile as tile
from concourse import mybir
from concourse.bass_utils import run_bass_kernel_spmd

F32 = mybir.dt.float32
F32R = mybir.dt.float32r
BF16 = mybir.dt.bfloat16
AF = mybir.ActivationFunctionType
ALU = mybir.AluOpType
AX = mybir.AxisListType

N_CORES = 8
B, C, P = 256, 128, 27
NTOK = P * P          # 729
S = B // N_CORES      # 32 samples per core
NT = 6                # token tiles: 5*128 + 89
TOK_SIZES = [128, 128, 128, 128, 128, 89]
CGRP = 4              # conv2 sample-group size


def _split_waits(nc, maxw=1):
    """walrus CoreV3 rejects instructions carrying >1 sem-wait; hoist
    extras onto preceding same-engine no-op carriers."""
    import bass_rust

    for bb in nc.m.functions[0].blocks:
        newlist = []
        for ins in bb.instructions:
            sw = ins.sync_info
            if sw and sw.on_wait and len(sw.on_wait) > maxw:
                waits = list(sw.on_wait)
                keep = waits[-maxw:]
                hoist = waits[:-maxw]
                for i in range(0, len(hoist), maxw):
                    chunk = hoist[i : i + maxw]
                    nop = bass_rust.InstNoOp(
                        name=f"{ins.name}_wsplit{i}", ins=[], outs=[]
                    )
                    nop.engine = ins.engine
                    nop.sync_info = mybir.SyncInfo(on_wait=list(chunk), on_update=[])
                    nc.register_instruction(nop, overwrite=True)
                    newlist.append(nop)
                ins.sync_info = mybir.SyncInfo(
                    on_wait=list(keep), on_update=list(sw.on_update)
                )
            newlist.append(ins)
        bb.instructions[:] = newlist


def _prep_weights(inputs):
    """Host-side weight massaging (all cheap numpy)."""
    f = lambda a: np.ascontiguousarray(np.asarray(a, np.float32))
    w_qkv = f(inputs["w_qkv"]).copy()          # [128, 384]
    b_qkv = f(inputs["b_qkv"]).copy()          # [384]
    # fold channel-mean subtraction of q and k into the weights/bias
    w_qkv[:, 0:128] -= w_qkv[:, 0:128].mean(axis=1, keepdims=True)
    w_qkv[:, 128:256] -= w_qkv[:, 128:256].mean(axis=1, keepdims=True)
    b_qkv[0:128] -= b_qkv[0:128].mean()
    b_qkv[128:256] -= b_qkv[128:256].mean()

    w_ffn1 = f(inputs["w_ffn1"]).reshape(64, 256)     # [out, in]
    w1x = np.ascontiguousarray(w_ffn1[:, 0:128].T)    # [128, 64]
    w1a = np.ascontiguousarray(w_ffn1[:, 128:256].T)  # [128, 64]
    w2t = np.ascontiguousarray(f(inputs["w_ffn2"]).reshape(128, 64).T)  # [64, 128]

    # conv taps -> [in_ch, 9, out_ch]
    wc1 = np.ascontiguousarray(
        f(inputs["w_c1"]).transpose(2, 3, 1, 0).reshape(9, 128, 64).transpose(1, 0, 2)
    )  # [128, 9, 64]
    wc2 = np.ascontiguousarray(
        f(inputs["w_c2"]).transpose(2, 3, 1, 0).reshape(9, 64, 128).transpose(1, 0, 2)
    )  # [64, 9, 128]

    w1r = np.ascontiguousarray(f(inputs["w_fc1"]).reshape(128, 25, 512))
    wf2 = np.ascontiguousarray(f(inputs["w_fc2"]).reshape(4, 128, 512).transpose(1, 0, 2))
    wcls = np.ascontiguousarray(f(inputs["w_cls"]).reshape(4, 128, 16).transpose(1, 0, 2))

    col = lambda a: np.ascontiguousarray(f(a).reshape(-1, 1))
    row = lambda a: np.ascontiguousarray(f(a).reshape(1, -1))
    w = {
        "wqkv": w_qkv,
        "bqkv_row": row(b_qkv),
        "wln": f(inputs["w_ln"]),
        "bln": col(inputs["b_ln"]),
        "w1x": w1x,
        "w1a": w1a,
        "b1": col(inputs["b_ffn1"]),
        "w2t": w2t,
        "b2": col(inputs["b_ffn2"]),
        "wc1": wc1,
        "bc1": col(inputs["b_c1"]),
        "wc2": wc2,
        "bc2": col(inputs["b_c2"]),
        "w1r": w1r,
        "b1row": row(inputs["b_fc1"]),
        "wf2": wf2,
        "b2row": row(inputs["b_fc2"]),
        "wcls": wcls,
        "bcrow": row(inputs["b_cls"]),
        "eye": np.eye(128, dtype=np.float32),
        "ones1": np.ones((1, S), dtype=np.float32),
    }
    flags = {
        "qkv_bias": bool(np.any(b_qkv)),
        "fc1_bias": bool(np.any(w["b1row"])),
        "fc2_bias": bool(np.any(w["b2row"])),
        "cls_bias": bool(np.any(w["bcrow"])),
    }
    return w, flags


class _W:
    pass


_F32_WEIGHTS = {"bln", "b1", "b2", "bc1", "bc2"}  # activation-bias operands
_BF16_WEIGHTS = {"wln", "w1a", "w2t", "wc1", "wc2"}  # bf16 matmul path


def _load_weights(nc, pool, wvals):
    """Declare dram params + DMA every weight into resident SBUF tiles.
    bf16 weights are cast during a gpsimd DMA (only engine that casts)."""
    W = _W()
    for name, arr in wvals.items():
        if name in _F32_WEIGHTS:
            dt = F32
        elif name in _BF16_WEIGHTS:
            dt = BF16
        else:
            dt = F32R
        dram = nc.declare_dram_parameter(
            name, list(arr.shape), F32 if dt == BF16 else dt, isOutput=False
        )
        t = pool.tile(list(arr.shape), dt, name=f"sb_{name}")
        if dt == BF16:
            nc.gpsimd.dma_start(out=t, in_=dram[:])
        else:
            nc.sync.dma_start(out=t, in_=dram[:])
        setattr(W, name, t)
    return W


def _win(ap, offset, dims):
    """Manual sub-AP of a tile: dims = [[stride, count], ...] free dims.
    Rows may overlap (fp32r moving operands need inner size %4==0, so
    windows are padded to 28/12 wide and garbage columns dropped later)."""
    return bass.AP(
        tensor=ap.tensor, offset=ap.offset + offset,
        ap=[list(ap.ap[0])] + [list(d) for d in dims],
    )


def _mm(nc, out, lhsT, rhs, start=True, stop=True):
    nc.tensor.matmul(out, lhsT, rhs, start=start, stop=stop)


def _tp(nc, out, in_, eye):
    nc.tensor.matmul(
        out.bitcast(in_.dtype), in_, eye.bitcast(in_.dtype), is_transpose=True
    )


def _emit_sample(nc, pools, W, flags, x_dram, s, O2buf, grp, taps):
    acts, stats, psum = pools["acts"], pools["stats"], pools["psum"]

    x_s = acts.tile([C, 768], F32R, name="x_s")
    nc.sync.dma_start(out=x_s[:, 0:NTOK], in_=x_dram[s])

    # ---- qkv (token-major), squares + sums ----
    q2 = acts.tile([128, NT, 128], F32, name="q2")
    k2 = acts.tile([128, NT, 128], F32, name="k2")
    kvsrc = acts.tile([128, NT, 256], F32R, name="kvsrc")  # [v | k2a]
    for half in range(2):
        pq = psum.tile([128, 3, 512], F32, name="pq", tag="pqkv", bufs=1)
        for i in range(3):
            t = half * 3 + i
            nt = TOK_SIZES[t]
            _mm(nc, pq[0:nt, i, 0:384], x_s[:, 128 * t : 128 * t + nt], W.wqkv,
                start=True, stop=not flags["qkv_bias"])
            if flags["qkv_bias"]:
                _mm(nc, pq[0:nt, i, 0:384], W.ones1[0:1, 0:nt], W.bqkv_row,
                    start=False, stop=True)
        h3 = slice(3 * half, 3 * half + 3)
        nc.scalar.activation(q2[:, h3, :], pq[:, :, 0:128], AF.Square)
        nc.scalar.activation(k2[:, h3, :], pq[:, :, 128:256], AF.Square)
        nc.vector.tensor_copy(kvsrc[:, h3, 0:128], pq[:, :, 256:384])

    sq2 = stats.tile([128, NT], F32, name="sq2")
    sk2 = stats.tile([128, NT], F32, name="sk2")
    sq4 = stats.tile([128, NT], F32, name="sq4")
    q4s = acts.tile([128, 128], F32, name="q4s")
    nc.vector.reduce_sum(sq2, q2, axis=AX.X)
    nc.vector.reduce_sum(sk2, k2, axis=AX.X)
    for t in range(NT):
        nc.vector.scalar_tensor_tensor(
            out=q4s, in0=q2[:, t, :], scalar=1.0, in1=q2[:, t, :],
            op0=ALU.mult, op1=ALU.mult, accum_out=sq4[:, t : t + 1],
        )

    # per-token scalars: s1q = 1/(sq2+1e-7); cq = s1q / max(s1q*sqrt(sq4), 1e-12)
    s1q = stats.tile([128, NT], F32, name="s1q")
    nc.vector.tensor_scalar_add(s1q, sq2, 1e-7)
    nc.vector.reciprocal(s1q, s1q)
    nq = stats.tile([128, NT], F32, name="nq")
    nc.scalar.activation(nq, sq4, AF.Sqrt)
    cq = stats.tile([128, NT], F32, name="cq")
    nc.vector.tensor_mul(cq, s1q, nq)
    nc.vector.tensor_scalar_max(cq, cq, 1e-12)
    nc.vector.reciprocal(cq, cq)
    nc.vector.tensor_mul(cq, cq, s1q)
    s1k = stats.tile([128, NT], F32, name="s1k")
    nc.vector.tensor_scalar_add(s1k, sk2, 1e-7)
    nc.vector.reciprocal(s1k, s1k)

    # q2n = q2 * cq (per-token row scale); k2a = k2 * s1k
    q2n = acts.tile([128, NT, 128], F32R, name="q2n")
    for t in range(NT):
        nc.gpsimd.tensor_scalar_mul(q2n[:, t, :], q2[:, t, :], cq[:, t : t + 1])
    for t in range(NT):
        nc.gpsimd.tensor_scalar_mul(
            kvsrc[:, t, 128:256], k2[:, t, :], s1k[:, t : t + 1]
        )

    # ---- kv & gram: psum [c, 0:128]=k2a^T v, [c,128:256]=k2a^T k2a ----
    pkv = psum.tile([128, 512], F32, name="pkv", tag="ps1", bufs=1)
    for t in range(NT):
        nt = TOK_SIZES[t]
        _mm(nc, pkv[:, 0:256], kvsrc[0:nt, t, 128:256], kvsrc[0:nt, t, :],
            start=(t == 0), stop=(t == NT - 1))
    tmpd = acts.tile([128, 128], F32, name="tmpd")
    s2 = stats.tile([128, 1], F32, name="s2")
    nc.vector.tensor_mul(tmpd, pkv[:, 128:256], W.eye)
    nc.vector.reduce_sum(s2, tmpd, axis=AX.X)
    # invs27 = 1/(27*max(sqrt(s2), 1e-12)) ; folds the 1/sqrt(N) of t2
    invs = stats.tile([128, 1], F32, name="invs")
    nc.scalar.activation(invs, s2, AF.Sqrt, scale=float(NTOK))  # 27*sqrt(s2)
    nc.vector.tensor_scalar_max(invs, invs, 27e-12)
    nc.vector.reciprocal(invs, invs)
    kvsb = acts.tile([128, 128], F32R, name="kvsb")
    nc.scalar.mul(kvsb, pkv[:, 0:128], invs)

    # ---- transpose q2n to channel-major ----
    pqt = psum.tile([128, 768], F32, name="pqt", tag="ps2", bufs=2)
    for t in range(NT):
        # full 128-row transpose even for the 89-token tail tile: fp32r
        # transpose mode rejects non-multiple-of-4 sizes; the garbage
        # columns land past token 729 and are never read.
        _tp(nc, pqt[:, 128 * t : 128 * (t + 1)], q2n[:, t, :], W.eye)
    q2nT = acts.tile([128, 768], F32R, name="q2nT")
    nc.vector.tensor_copy(q2nT, pqt)

    # ---- vt = v + t2, both accumulated in PSUM (channel-major) ----
    wv = W.wqkv[:, 256:384]
    pt2 = psum.tile([128, 768], F32, name="pt2", tag="ps2", bufs=2)
    _mm(nc, pt2[:, 0:512], kvsb, q2nT[:, 0:512], start=True, stop=False)
    _mm(nc, pt2[:, 0:512], wv, x_s[:, 0:512], start=False, stop=True)
    _mm(nc, pt2[:, 512:768], kvsb, q2nT[:, 512:768], start=True, stop=False)
    _mm(nc, pt2[:, 512:768], wv, x_s[:, 512:768], start=False, stop=True)
    vt = acts.tile([C, 732], BF16, name="vt")
    nc.scalar.copy(vt[:, 0:NTOK], pt2[:, 0:729])

    # ---- attn = w_ln^T @ vt + b_ln (single bf16 matmul) ----
    pat = psum.tile([128, 768], F32, name="pat", tag="ps2", bufs=2)
    _mm(nc, pat[:, 0:512], W.wln, vt[:, 0:512])
    _mm(nc, pat[:, 512:732], W.wln, vt[:, 512:732])
    attn = acts.tile([C, 732], BF16, name="attn")
    nc.scalar.activation(attn[:, 0:NTOK], pat[:, 0:729], AF.Identity, bias=W.bln)

    # ---- ffn1: h = lrelu(w1x^T x + w1a^T attn + b1) ----
    ph = psum.tile([64, 768], F32, name="ph", tag="ps2", bufs=2)
    _mm(nc, ph[:, 0:512], W.w1x, x_s[:, 0:512], start=True, stop=False)
    _mm(nc, ph[:, 512:768], W.w1x, x_s[:, 512:768], start=True, stop=False)
    _mm(nc, ph[:, 0:512], W.w1a, attn[:, 0:512], start=False, stop=True)
    _mm(nc, ph[:, 512:732], W.w1a, attn[:, 512:732], start=False, stop=True)
    h0 = acts.tile([64, NTOK], F32, name="h0")
    nc.scalar.activation(h0, ph[:, 0:729], AF.Identity, bias=W.b1)
    h = acts.tile([64, 732], BF16, name="h")
    nc.vector.scalar_tensor_tensor(
        out=h[:, 0:NTOK], in0=h0, scalar=0.01, in1=h0, op0=ALU.mult, op1=ALU.max
    )

    # ---- ffn2 + residual (residual via identity matmul): ----
    pxen = psum.tile([128, 768], F32, name="pxen", tag="ps2", bufs=2)
    _mm(nc, pxen[:, 0:512], W.w2t, h[:, 0:512], start=True, stop=False)
    _mm(nc, pxen[:, 512:732], W.w2t, h[:, 512:732], start=True, stop=False)
    _mm(nc, pxen[:, 0:512], W.eye, x_s[:, 0:512], start=False, stop=True)
    _mm(nc, pxen[:, 512:768], W.eye, x_s[:, 512:768], start=False, stop=True)
    xen = acts.tile([C, 768], BF16, name="xen")
    nc.scalar.activation(xen[:, 0:NTOK], pxen[:, 0:729], AF.Identity, bias=W.b2)

    # ---- conv1 3x3 128->64 on 27x27 -> 25x25, relu, pool -> 12x12 ----
    pc1a = psum.tile([64, 13, 28], F32, name="pc1a", tag="ps1", bufs=1)
    pc1b = psum.tile([64, 12, 28], F32, name="pc1b", tag="ps2", bufs=2)
    for ky in range(3):
        for kx in range(3):
            tap = ky * 3 + kx
            # 28-wide overlapping windows; 3 garbage cols/row dropped later
            _mm(nc, pc1a, W.wc1[:, tap, :],
                _win(xen, ky * 27 + kx, [[27, 13], [1, 28]]),
                start=(tap == 0), stop=(tap == 8))
            _mm(nc, pc1b, W.wc1[:, tap, :],
                _win(xen, (ky + 13) * 27 + kx, [[27, 12], [1, 28]]),
                start=(tap == 0), stop=(tap == 8))
    o1r = acts.tile([64, 625], BF16, name="o1r")
    o1rv = o1r.rearrange("p (h w) -> p h w", h=25)
    nc.scalar.activation(o1rv[:, 0:13, :], pc1a[:, :, 0:25], AF.Relu, bias=W.bc1)
    nc.scalar.activation(o1rv[:, 13:25, :], pc1b[:, :, 0:25], AF.Relu, bias=W.bc1)
    o1r3 = o1r.rearrange("p (h w) -> p h w", h=25)
    m1 = acts.tile([64, 144], BF16, name="m1")
    m1v = m1.rearrange("p (a b) -> p a b", a=12)
    m2 = acts.tile([64, 144], BF16, name="m2")
    m2v = m2.rearrange("p (a b) -> p a b", a=12)
    g = s % CGRP
    o1pv = grp["o1p"][:, g, 0:144].rearrange("p (a b) -> p a b", a=12)
    nc.vector.tensor_max(m1v, o1r3[:, 0:24:2, 0:24:2], o1r3[:, 0:24:2, 1:25:2])
    nc.vector.tensor_max(m2v, o1r3[:, 1:25:2, 0:24:2], o1r3[:, 1:25:2, 1:25:2])
    nc.vector.tensor_max(o1pv, m1v, m2v)

    if taps is not None and s == 0:
        for nm, t in (
            ("q2n", q2n), ("kvsb", kvsb), ("attn", attn), ("xen", xen),
            ("o1p", grp["o1p"][:, 0, :]), ("vt", vt), ("q2", q2),
        ):
            d = nc.declare_dram_parameter(f"tap_{nm}", list(t.shape), t.dtype, isOutput=True)
            nc.sync.dma_start(out=d[:], in_=t)
            taps.append(f"tap_{nm}")


def _emit_conv2_group(nc, pools, W, O2buf, grp, g0, gn):
    """conv2+pool for a group of gn samples (moving dim = gn*100)."""
    acts, psum = pools["acts"], pools["psum"]
    pc2 = psum.tile([128, CGRP, 10, 12], F32, name="pc2", tag="ps1", bufs=1)
    for ky in range(3):
        for kx in range(3):
            tap = ky * 3 + kx
            # 12-wide overlapping windows (inner %4); 2 garbage cols/row
            _mm(nc, pc2[:, 0:gn], W.wc2[:, tap, :],
                _win(grp["o1p"], ky * 12 + kx, [[148, gn], [12, 10], [1, 12]]),
                start=(tap == 0), stop=(tap == 8))
    o2r = acts.tile([128, CGRP, 100], BF16, name="o2r")
    o2rv = o2r.rearrange("p g (h w) -> p g h w", h=10)
    nc.scalar.activation(o2rv[:, 0:gn], pc2[:, 0:gn, :, 0:10], AF.Relu, bias=W.bc2)
    o2r4 = o2rv
    n1 = acts.tile([128, CGRP, 25], F32, name="n1")
    n1v = n1.rearrange("p g (a b) -> p g a b", a=5)
    n2 = acts.tile([128, CGRP, 25], F32, name="n2")
    n2v = n2.rearrange("p g (a b) -> p g a b", a=5)
    nc.vector.tensor_max(
        n1v[:, 0:gn], o2r4[:, 0:gn, 0:10:2, 0:10:2], o2r4[:, 0:gn, 0:10:2, 1:10:2]
    )
    nc.vector.tensor_max(
        n2v[:, 0:gn], o2r4[:, 0:gn, 1:10:2, 0:10:2], o2r4[:, 0:gn, 1:10:2, 1:10:2]
    )
    outv = (
        O2buf[:, :, g0 : g0 + gn]
        .rearrange("p a g -> p g a")
        .rearrange("p g (a b) -> p g a b", a=5)
    )
    nc.vector.tensor_max(outv, n1v[:, 0:gn], n2v[:, 0:gn])


def _emit_fc(nc, pools, W, flags, out_dram, O2buf, ns):
    psum, fc = pools["psum"], pools["fc"]
    ones = W.ones1[0:1, 0:ns]

    po3 = psum.tile([ns, 512], F32, name="po3", tag="ps1", bufs=1)
    for p in range(25):
        _mm(nc, po3, O2buf[:, p, :], W.w1r[:, p, :],
            start=(p == 0), stop=(p == 24 and not flags["fc1_bias"]))
    if flags["fc1_bias"]:
        _mm(nc, po3, ones, W.b1row, start=False, stop=True)
    o3r = fc.tile([ns, 512], F32R, name="o3r")
    nc.scalar.activation(o3r, po3, AF.Relu)

    po3t = psum.tile([128, 4, ns], F32, name="po3t", tag="ps1", bufs=1)
    for j in range(4):
        _tp(nc, po3t[:, j, :], o3r[:, 128 * j : 128 * (j + 1)], W.eye[0:ns, 0:ns])
    o3T = fc.tile([128, 4, ns], F32R, name="o3T")
    nc.vector.tensor_copy(o3T, po3t)

    po4 = psum.tile([ns, 512], F32, name="po4", tag="ps1", bufs=1)
    for j in range(4):
        _mm(nc, po4, o3T[:, j, :], W.wf2[:, j, :],
            start=(j == 0), stop=(j == 3 and not flags["fc2_bias"]))
    if flags["fc2_bias"]:
        _mm(nc, po4, ones, W.b2row, start=False, stop=True)
    o4r = fc.tile([ns, 512], F32R, name="o4r")
    nc.scalar.activation(o4r, po4, AF.Relu)

    po4t = psum.tile([128, 4, ns], F32, name="po4t", tag="ps1", bufs=1)
    for j in range(4):
        _tp(nc, po4t[:, j, :], o4r[:, 128 * j : 128 * (j + 1)], W.eye[0:ns, 0:ns])
    o4T = fc.tile([128, 4, ns], F32R, name="o4T")
    nc.vector.tensor_copy(o4T, po4t)

    pcls = psum.tile([ns, 512], F32, name="pcls", tag="ps1", bufs=1)
    for j in range(4):
        _mm(nc, pcls[:, 0:16], o4T[:, j, :], W.wcls[:, j, :],
            start=(j == 0), stop=(j == 3 and not flags["cls_bias"]))
    if flags["cls_bias"]:
        _mm(nc, pcls[:, 0:16], ones, W.bcrow, start=False, stop=True)
    outsb = fc.tile([ns, 16], F32, name="outsb")
    nc.vector.tensor_copy(outsb, pcls[:, 0:16])
    nc.sync.dma_start(out=out_dram[:], in_=outsb)


def build_nc(wvals, flags, n_samples=S, debug=False):
    nc = bass.Bass()
    x_dram = nc.declare_dram_parameter("x", [n_samples, C, NTOK], F32R, isOutput=False)
    out_dram = nc.declare_dram_parameter("out", [n_samples, 16], F32, isOutput=True)
    taps = [] if debug else None

    with tile.TileContext(nc) as tc:
        with (
            tc.tile_pool(name="wts", bufs=1) as wts,
            tc.tile_pool(name="acts", bufs=2) as acts,
            tc.tile_pool(name="stats", bufs=3) as stats,
            tc.tile_pool(name="fc", bufs=1) as fc,
            tc.tile_pool(name="psum", bufs=1, space="PSUM") as psum,
        ):
            pools = {"acts": acts, "stats": stats, "psum": psum, "fc": fc}
            W = _load_weights(nc, wts, wvals)
            O2buf = fc.tile([128, 25, n_samples], F32R, name="O2buf")
            grp = {}
            for s in range(n_samples):
                if s % CGRP == 0:
                    grp["o1p"] = acts.tile([64, CGRP, 148], BF16, name="o1p_grp")
                _emit_sample(nc, pools, W, flags, x_dram, s, O2buf, grp, taps)
                if s % CGRP == CGRP - 1 or s == n_samples - 1:
                    g0 = (s // CGRP) * CGRP
                    _emit_conv2_group(nc, pools, W, O2buf, grp, g0, s - g0 + 1)
            _emit_fc(nc, pools, W, flags, out_dram, O2buf, n_samples)

    _split_waits(nc)
    return nc, taps


_BUILD_CACHE = {}


def kernel(**inputs):
    wvals, flags = _prep_weights(inputs)
    key = tuple(sorted(flags.items()))
    if key not in _BUILD_CACHE:
        _BUILD_CACHE[key] = build_nc(wvals, flags)
    nc, _ = _BUILD_CACHE[key]

    x = np.ascontiguousarray(np.asarray(inputs["x"], np.float32)).reshape(
        N_CORES, S, C, NTOK
    )
    in_maps = []
    for c in range(N_CORES):
        m = {"x": np.ascontiguousarray(x[c])}
        m.update(wvals)
        in_maps.append(m)
    last_err = None
    for _attempt in range(3):
        try:
            res = run_bass_kernel_spmd(nc, in_maps, core_ids=list(range(N_CORES)))
            break
        except Exception as e:  # transient device faults: retry
            last_err = e
    else:
        raise last_err
    out = np.concatenate([res.results[c]["out"] for c in range(N_CORES)], axis=0)
    return out.astype(np.float32)


# revision 15
# speedup vs baseline: 1.6823x; 1.6823x over previous
"""Trainium2 Bass kernel for nn_Discriminator_AddDim_ESSAAttn.

Network (per sample, C=128, 27x27 spatial, N=729 tokens):
  ESSA linear attention -> concat -> 1x1-conv FFN (+residual) ->
  3x3 conv/relu/pool x2 -> 3 FC layers -> [16] logits.
Batch 256 is sharded 32-per-core across 8 NeuronCores (pure data
parallel, weights replicated). Everything for one sample lives in SBUF;
weights are resident.

Layout strategy:
  - qkv computed token-major ([tokens, 384] psum tiles) so per-token
    stats (channel means/sums/norms) are per-partition scalars.
    Channel-mean subtraction is folded into the qkv weights host-side
    (centering is linear: w_q' = w_q - mean_j(w_q)).
  - q2-row-normalisation folded into per-partition tensor_scalar muls;
    k2 column (token-dim) L2 norm folded into a post-matmul per-channel
    scale using the gram matrix diag computed in the same PE pass.
  - kv/t2/attn/convs run channel-major; q2n is PE-transposed.
  - v (channel-major) and the ffn2 residual are accumulated directly in
    PSUM by extra matmuls (w_v / identity as stationary).
  - convs are 9 accumulating matmuls over shifted [C, H, W] views;
    conv2 batches 4 samples per matmul to keep the moving dim >= 256.
  - all matmuls use float32r (single-pass full-rate fp32 on the PE;
    plain fp32 is a 2-pass 4 cyc/col emulation).
"""
import sys

sys.path.insert(0, "/opt/trn_rl_repo")

import numpy as np

import concourse.bass as bass
import concourse.tile as tile
from concourse import mybir
from concourse.bass_utils import run_bass_kernel_spmd

F32 = mybir.dt.float32
F32R = mybir.dt.float32r
BF16 = mybir.dt.bfloat16
AF = mybir.ActivationFunctionType
ALU = mybir.AluOpType
AX = mybir.AxisListType

N_CORES = 8
B, C, P = 256, 128, 27
NTOK = P * P          # 729
S = B // N_CORES      # 32 samples per core
NT = 6                # token tiles: 5*128 + 89
TOK_SIZES = [128, 128, 128, 128, 128, 89]
CGRP = 4              # conv2 sample-group size


def _split_waits(nc, maxw=1):
    """walrus CoreV3 rejects instructions carrying >1 sem-wait; hoist
    extras onto preceding same-engine no-op carriers."""
    import bass_rust

    for bb in nc.m.functions[0].blocks:
        newlist = []
        for ins in bb.instructions:
            sw = ins.sync_info
            if sw and sw.on_wait and len(sw.on_wait) > maxw:
                waits = list(sw.on_wait)
                keep = waits[-maxw:]
                hoist = waits[:-maxw]
                for i in range(0, len(hoist), maxw):
                    chunk = hoist[i : i + maxw]
                    nop = bass_rust.InstNoOp(
                        name=f"{ins.name}_wsplit{i}", ins=[], outs=[]
                    )
                    nop.engine = ins.engine
                    nop.sync_info = mybir.SyncInfo(on_wait=list(chunk), on_update=[])
                    nc.register_instruction(nop, overwrite=True)
                    newlist.append(nop)
                ins.sync_info = mybir.SyncInfo(
                    on_wait=list(keep), on_update=list(sw.on_update)
                )
            newlist.append(ins)
        bb.instructions[:] = newlist


def _prep_weights(inputs):
    """Host-side weight massaging (all cheap numpy)."""
    f = lambda a: np.ascontiguousarray(np.asarray(a, np.float32))
    w_qkv = f(inputs["w_qkv"]).copy()          # [128, 384]
    b_qkv = f(inputs["b_qkv"]).copy()          # [384]
    # fold channel-mean subtraction of q and k into the weights/bias
    w_qkv[:, 0:128] -= w_qkv[:, 0:128].mean(axis=1, keepdims=True)
    w_qkv[:, 128:256] -= w_qkv[:, 128:256].mean(axis=1, keepdims=True)
    b_qkv[0:128] -= b_qkv[0:128].mean()
    b_qkv[128:256] -= b_qkv[128:256].mean()

    w_ffn1 = f(inputs["w_ffn1"]).reshape(64, 256)     # [out, in]
    w1x = np.ascontiguousarray(w_ffn1[:, 0:128].T)    # [128, 64]
    w1a = np.ascontiguousarray(w_ffn1[:, 128:256].T)  # [128, 64]
    w2t = np.ascontiguousarray(f(inputs["w_ffn2"]).reshape(128, 64).T)  # [64, 128]

    # conv taps -> [in_ch, 9, out_ch]
    wc1 = np.ascontiguousarray(
        f(inputs["w_c1"]).transpose(2, 3, 1, 0).reshape(9, 128, 64).transpose(1, 0, 2)
    )  # [128, 9, 64]
    wc2 = np.ascontiguousarray(
        f(inputs["w_c2"]).transpose(2, 3, 1, 0).reshape(9, 64, 128).transpose(1, 0, 2)
    )  # [64, 9, 128]

    w1r = np.ascontiguousarray(f(inputs["w_fc1"]).reshape(128, 25, 512))
    wf2 = np.ascontiguousarray(f(inputs["w_fc2"]).reshape(4, 128, 512).transpose(1, 0, 2))
    wcls = np.ascontiguousarray(f(inputs["w_cls"]).reshape(4, 128, 16).transpose(1, 0, 2))

    col = lambda a: np.ascontiguousarray(f(a).reshape(-1, 1))
    row = lambda a: np.ascontiguousarray(f(a).reshape(1, -1))
    w = {
        "wqkv": w_qkv,
        "bqkv_row": row(b_qkv),
        "wln": f(inputs["w_ln"]),
        "bln": col(inputs["b_ln"]),
        "w1x": w1x,
        "w1a": w1a,
        "b1": col(inputs["b_ffn1"]),
        "w2t": w2t,
        "b2": col(inputs["b_ffn2"]),
        "wc1": wc1,
        "bc1": col(inputs["b_c1"]),
        "wc2": wc2,
        "bc2": col(inputs["b_c2"]),
        "w1r": w1r,
        "b1row": row(inputs["b_fc1"]),
        "wf2": wf2,
        "b2row": row(inputs["b_fc2"]),
        "wcls": wcls,
        "bcrow": row(inputs["b_cls"]),
        "eye": np.eye(128, dtype=np.float32),
        "ones1": np.ones((1, S), dtype=np.float32),
    }
    flags = {
        "qkv_bias": bool(np.any(b_qkv)),
        "fc1_bias": bool(np.any(w["b1row"])),
        "fc2_bias": bool(np.any(w["b2row"])),
        "cls_bias": bool(np.any(w["bcrow"])),
    }
    return w, flags


class _W:
    pass


_F32_WEIGHTS = {"bln", "b1", "b2", "bc1", "bc2"}  # activation-bias operands
_BF16_WEIGHTS = {"wln", "w1a", "w2t", "wc1", "wc2"}  # bf16 matmul path


def _load_weights(nc, pool, wvals):
    """Declare dram params + DMA every weight into resident SBUF tiles.
    bf16 weights are cast during a gpsimd DMA (only engine that casts)."""
    W = _W()
    for name, arr in wvals.items():
        if name in _F32_WEIGHTS:
            dt = F32
        elif name in _BF16_WEIGHTS:
            dt = BF16
        else:
            dt = F32R
        dram = nc.declare_dram_parameter(
            name, list(arr.shape), F32 if dt == BF16 else dt, isOutput=False
        )
        t = pool.tile(list(arr.shape), dt, name=f"sb_{name}")
        if dt == BF16:
            nc.gpsimd.dma_start(out=t, in_=dram[:])
        else:
            nc.sync.dma_start(out=t, in_=dram[:])
        setattr(W, name, t)
    return W


def _win(ap, offset, dims):
    """Manual sub-AP of a tile: dims = [[stride, count], ...] free dims.
    Rows may overlap (fp32r moving operands need inner size %4==0, so
    windows are padded to 28/12 wide and garbage columns dropped later)."""
    return bass.AP(
        tensor=ap.tensor, offset=ap.offset + offset,
        ap=[list(ap.ap[0])] + [list(d) for d in dims],
    )


def _mm(nc, out, lhsT, rhs, start=True, stop=True):
    nc.tensor.matmul(out, lhsT, rhs, start=start, stop=stop)


def _tp(nc, out, in_, eye):
    nc.tensor.matmul(
        out.bitcast(in_.dtype), in_, eye.bitcast(in_.dtype), is_transpose=True
    )


def _emit_sample(nc, pools, W, flags, x_dram, s, O2buf, grp, taps):
    acts, stats, psum = pools["acts"], pools["stats"], pools["psum"]

    x_s = acts.tile([C, 768], F32R, name="x_s")
    nc.sync.dma_start(out=x_s[:, 0:NTOK], in_=x_dram[s])

    # ---- qkv (token-major), squares + sums ----
    q2 = acts.tile([128, NT, 128], F32, name="q2")
    k2 = acts.tile([128, NT, 128], F32, name="k2")
    kvsrc = acts.tile([128, NT, 256], F32R, name="kvsrc")  # [v | k2a]
    for half in range(2):
        pq = psum.tile([128, 3, 512], F32, name="pq", tag="pqkv", bufs=1)
        for i in range(3):
            t = half * 3 + i
            nt = TOK_SIZES[t]
            _mm(nc, pq[0:nt, i, 0:384], x_s[:, 128 * t : 128 * t + nt], W.wqkv,
                start=True, stop=not flags["qkv_bias"])
            if flags["qkv_bias"]:
                _mm(nc, pq[0:nt, i, 0:384], W.ones1[0:1, 0:nt], W.bqkv_row,
                    start=False, stop=True)
        h3 = slice(3 * half, 3 * half + 3)
        nc.scalar.activation(q2[:, h3, :], pq[:, :, 0:128], AF.Square)
        nc.scalar.activation(k2[:, h3, :], pq[:, :, 128:256], AF.Square)
        nc.vector.tensor_copy(kvsrc[:, h3, 0:128], pq[:, :, 256:384])

    sq2 = stats.tile([128, NT], F32, name="sq2")
    sk2 = stats.tile([128, NT], F32, name="sk2")
    sq4 = stats.tile([128, NT], F32, name="sq4")
    q4s = acts.tile([128, 128], F32, name="q4s")
    nc.vector.reduce_sum(sq2, q2, axis=AX.X)
    nc.vector.reduce_sum(sk2, k2, axis=AX.X)
    for t in range(NT):
        nc.vector.scalar_tensor_tensor(
            out=q4s, in0=q2[:, t, :], scalar=1.0, in1=q2[:, t, :],
            op0=ALU.mult, op1=ALU.mult, accum_out=sq4[:, t : t + 1],
        )

    # per-token scalars: s1q = 1/(sq2+1e-7); cq = s1q / max(s1q*sqrt(sq4), 1e-12)
    s1q = stats.tile([128, NT], F32, name="s1q")
    nc.vector.tensor_scalar_add(s1q, sq2, 1e-7)
    nc.vector.reciprocal(s1q, s1q)
    nq = stats.tile([128, NT], F32, name="nq")
    nc.scalar.activation(nq, sq4, AF.Sqrt)
    cq = stats.tile([128, NT], F32, name="cq")
    nc.vector.tensor_mul(cq, s1q, nq)
    nc.vector.tensor_scalar_max(cq, cq, 1e-12)
    nc.vector.reciprocal(cq, cq)
    nc.vector.tensor_mul(cq, cq, s1q)
    s1k = stats.tile([128, NT], F32, name="s1k")
    nc.vector.tensor_scalar_add(s1k, sk2, 1e-7)
    nc.vector.reciprocal(s1k, s1k)

    # q2n = q2 * cq (per-token row scale); k2a = k2 * s1k
    q2n = acts.tile([128, NT, 128], F32R, name="q2n")
    for t in range(NT):
        nc.vector.tensor_scalar_mul(q2n[:, t, :], q2[:, t, :], cq[:, t : t + 1])
    for t in range(NT):
        nc.vector.tensor_scalar_mul(
            kvsrc[:, t, 128:256], k2[:, t, :], s1k[:, t : t + 1]
        )

    # ---- kv & gram: psum [c, 0:128]=k2a^T v, [c,128:256]=k2a^T k2a ----
    pkv = psum.tile([128, 512], F32, name="pkv", tag="ps1", bufs=1)
    for t in range(NT):
        nt = TOK_SIZES[t]
        _mm(nc, pkv[:, 0:256], kvsrc[0:nt, t, 128:256], kvsrc[0:nt, t, :],
            start=(t == 0), stop=(t == NT - 1))
    tmpd = acts.tile([128, 128], F32, name="tmpd")
    s2 = stats.tile([128, 1], F32, name="s2")
    nc.vector.tensor_mul(tmpd, pkv[:, 128:256], W.eye)
    nc.vector.reduce_sum(s2, tmpd, axis=AX.X)
    # invs27 = 1/(27*max(sqrt(s2), 1e-12)) ; folds the 1/sqrt(N) of t2
    invs = stats.tile([128, 1], F32, name="invs")
    nc.scalar.activation(invs, s2, AF.Sqrt, scale=float(NTOK))  # 27*sqrt(s2)
    nc.vector.tensor_scalar_max(invs, invs, 27e-12)
    nc.vector.reciprocal(invs, invs)
    kvsb = acts.tile([128, 128], F32R, name="kvsb")
    nc.scalar.mul(kvsb, pkv[:, 0:128], invs)

    # ---- transpose q2n to channel-major ----
    pqt = psum.tile([128, 768], F32, name="pqt", tag="ps2", bufs=2)
    for t in range(NT):
        # full 128-row transpose even for the 89-token tail tile: fp32r
        # transpose mode rejects non-multiple-of-4 sizes; the garbage
        # columns land past token 729 and are never read.
        _tp(nc, pqt[:, 128 * t : 128 * (t + 1)], q2n[:, t, :], W.eye)
    q2nT = acts.tile([128, 768], F32R, name="q2nT")
    nc.vector.tensor_copy(q2nT, pqt)

    # ---- vt = v + t2, both accumulated in PSUM (channel-major) ----
    wv = W.wqkv[:, 256:384]
    pt2 = psum.tile([128, 768], F32, name="pt2", tag="ps2", bufs=2)
    _mm(nc, pt2[:, 0:512], kvsb, q2nT[:, 0:512], start=True, stop=False)
    _mm(nc, pt2[:, 0:512], wv, x_s[:, 0:512], start=False, stop=True)
    _mm(nc, pt2[:, 512:768], kvsb, q2nT[:, 512:768], start=True, stop=False)
    _mm(nc, pt2[:, 512:768], wv, x_s[:, 512:768], start=False, stop=True)
    vt = acts.tile([C, 732], BF16, name="vt")
    nc.scalar.copy(vt[:, 0:NTOK], pt2[:, 0:729])

    # ---- attn = w_ln^T @ vt + b_ln (single bf16 matmul) ----
    pat = psum.tile([128, 768], F32, name="pat", tag="ps2", bufs=2)
    _mm(nc, pat[:, 0:512], W.wln, vt[:, 0:512])
    _mm(nc, pat[:, 512:732], W.wln, vt[:, 512:732])
    attn = acts.tile([C, 732], BF16, name="attn")
    nc.scalar.activation(attn[:, 0:NTOK], pat[:, 0:729], AF.Identity, bias=W.bln)

    # ---- ffn1: h = lrelu(w1x^T x + w1a^T attn + b1) ----
    ph = psum.tile([64, 768], F32, name="ph", tag="ps2", bufs=2)
    _mm(nc, ph[:, 0:512], W.w1x, x_s[:, 0:512], start=True, stop=False)
    _mm(nc, ph[:, 512:768], W.w1x, x_s[:, 512:768], start=True, stop=False)
    _mm(nc, ph[:, 0:512], W.w1a, attn[:, 0:512], start=False, stop=True)
    _mm(nc, ph[:, 512:732], W.w1a, attn[:, 512:732], start=False, stop=True)
    h0 = acts.tile([64, NTOK], F32, name="h0")
    nc.scalar.activation(h0, ph[:, 0:729], AF.Identity, bias=W.b1)
    h = acts.tile([64, 732], BF16, name="h")
    nc.vector.scalar_tensor_tensor(
        out=h[:, 0:NTOK], in0=h0, scalar=0.01, in1=h0, op0=ALU.mult, op1=ALU.max
    )

    # ---- ffn2 + residual (residual via identity matmul): ----
    pxen = psum.tile([128, 768], F32, name="pxen", tag="ps2", bufs=2)
    _mm(nc, pxen[:, 0:512], W.w2t, h[:, 0:512], start=True, stop=False)
    _mm(nc, pxen[:, 512:732], W.w2t, h[:, 512:732], start=True, stop=False)
    _mm(nc, pxen[:, 0:512], W.eye, x_s[:, 0:512], start=False, stop=True)
    _mm(nc, pxen[:, 512:768], W.eye, x_s[:, 512:768], start=False, stop=True)
    xen = acts.tile([C, 768], BF16, name="xen")
    nc.scalar.activation(xen[:, 0:NTOK], pxen[:, 0:729], AF.Identity, bias=W.b2)

    # ---- conv1 3x3 128->64 on 27x27 -> 25x25, relu, pool -> 12x12 ----
    pc1a = psum.tile([64, 13, 28], F32, name="pc1a", tag="ps1", bufs=1)
    pc1b = psum.tile([64, 12, 28], F32, name="pc1b", tag="ps2", bufs=2)
    for ky in range(3):
        for kx in range(3):
            tap = ky * 3 + kx
            # 28-wide overlapping windows; 3 garbage cols/row dropped later
            _mm(nc, pc1a, W.wc1[:, tap, :],
                _win(xen, ky * 27 + kx, [[27, 13], [1, 28]]),
                start=(tap == 0), stop=(tap == 8))
            _mm(nc, pc1b, W.wc1[:, tap, :],
                _win(xen, (ky + 13) * 27 + kx, [[27, 12], [1, 28]]),
                start=(tap == 0), stop=(tap == 8))
    o1r = acts.tile([64, 625], BF16, name="o1r")
    o1rv = o1r.rearrange("p (h w) -> p h w", h=25)
    nc.scalar.activation(o1rv[:, 0:13, :], pc1a[:, :, 0:25], AF.Relu, bias=W.bc1)
    nc.scalar.activation(o1rv[:, 13:25, :], pc1b[:, :, 0:25], AF.Relu, bias=W.bc1)
    o1r3 = o1r.rearrange("p (h w) -> p h w", h=25)
    m1 = acts.tile([64, 144], BF16, name="m1")
    m1v = m1.rearrange("p (a b) -> p a b", a=12)
    m2 = acts.tile([64, 144], BF16, name="m2")
    m2v = m2.rearrange("p (a b) -> p a b", a=12)
    g = s % CGRP
    o1pv = grp["o1p"][:, g, 0:144].rearrange("p (a b) -> p a b", a=12)
    nc.vector.tensor_max(m1v, o1r3[:, 0:24:2, 0:24:2], o1r3[:, 0:24:2, 1:25:2])
    nc.vector.tensor_max(m2v, o1r3[:, 1:25:2, 0:24:2], o1r3[:, 1:25:2, 1:25:2])
    nc.vector.tensor_max(o1pv, m1v, m2v)

    if taps is not None and s == 0:
        for nm, t in (
            ("q2n", q2n), ("kvsb", kvsb), ("attn", attn), ("xen", xen),
            ("o1p", grp["o1p"][:, 0, :]), ("vt", vt), ("q2", q2),
        ):
            d = nc.declare_dram_parameter(f"tap_{nm}", list(t.shape), t.dtype, isOutput=True)
            nc.sync.dma_start(out=d[:], in_=t)
            taps.append(f"tap_{nm}")


def _emit_conv2_group(nc, pools, W, O2buf, grp, g0, gn):
    """conv2+pool for a group of gn samples (moving dim = gn*100)."""
    acts, psum = pools["acts"], pools["psum"]
    pc2 = psum.tile([128, CGRP, 10, 12], F32, name="pc2", tag="ps1", bufs=1)
    for ky in range(3):
        for kx in range(3):
            tap = ky * 3 + kx
            # 12-wide overlapping windows (inner %4); 2 garbage cols/row
            _mm(nc, pc2[:, 0:gn], W.wc2[:, tap, :],
                _win(grp["o1p"], ky * 12 + kx, [[148, gn], [12, 10], [1, 12]]),
                start=(tap == 0), stop=(tap == 8))
    o2r = acts.tile([128, CGRP, 100], BF16, name="o2r")
    o2rv = o2r.rearrange("p g (h w) -> p g h w", h=10)
    nc.scalar.activation(o2rv[:, 0:gn], pc2[:, 0:gn, :, 0:10], AF.Relu, bias=W.bc2)
    o2r4 = o2rv
    n1 = acts.tile([128, CGRP, 25], F32, name="n1")
    n1v = n1.rearrange("p g (a b) -> p g a b", a=5)
    n2 = acts.tile([128, CGRP, 25], F32, name="n2")
    n2v = n2.rearrange("p g (a b) -> p g a b", a=5)
    nc.vector.tensor_max(
        n1v[:, 0:gn], o2r4[:, 0:gn, 0:10:2, 0:10:2], o2r4[:, 0:gn, 0:10:2, 1:10:2]
    )
    nc.vector.tensor_max(
        n2v[:, 0:gn], o2r4[:, 0:gn, 1:10:2, 0:10:2], o2r4[:, 0:gn, 1:10:2, 1:10:2]
    )
    outv = (
        O2buf[:, :, g0 : g0 + gn]
        .rearrange("p a g -> p g a")
        .rearrange("p g (a b) -> p g a b", a=5)
    )
    nc.vector.tensor_max(outv, n1v[:, 0:gn], n2v[:, 0:gn])


def _emit_fc(nc, pools, W, flags, out_dram, O2buf, ns):
    psum, fc = pools["psum"], pools["fc"]
    ones = W.ones1[0:1, 0:ns]

    po3 = psum.tile([ns, 512], F32, name="po3", tag="ps1", bufs=1)
    for p in range(25):
        _mm(nc, po3, O2buf[:, p, :], W.w1r[:, p, :],
            start=(p == 0), stop=(p == 24 and not flags["fc1_bias"]))
    if flags["fc1_bias"]:
        _mm(nc, po3, ones, W.b1row, start=False, stop=True)
    o3r = fc.tile([ns, 512], F32R, name="o3r")
    nc.scalar.activation(o3r, po3, AF.Relu)

    po3t = psum.tile([128, 4, ns], F32, name="po3t", tag="ps1", bufs=1)
    for j in range(4):
        _tp(nc, po3t[:, j, :], o3r[:, 128 * j : 128 * (j + 1)], W.eye[0:ns, 0:ns])
    o3T = fc.tile([128, 4, ns], F32R, name="o3T")
    nc.vector.tensor_copy(o3T, po3t)

    po4 = psum.tile([ns, 512], F32, name="po4", tag="ps1", bufs=1)
    for j in range(4):
        _mm(nc, po4, o3T[:, j, :], W.wf2[:, j, :],
            start=(j == 0), stop=(j == 3 and not flags["fc2_bias"]))
    if flags["fc2_bias"]:
        _mm(nc, po4, ones, W.b2row, start=False, stop=True)
    o4r = fc.tile([ns, 512], F32R, name="o4r")
    nc.scalar.activation(o4r, po4, AF.Relu)

    po4t = psum.tile([128, 4, ns], F32, name="po4t", tag="ps1", bufs=1)
    for j in range(4):
        _tp(nc, po4t[:, j, :], o4r[:, 128 * j : 128 * (j + 1)], W.eye[0:ns, 0:ns])
    o4T = fc.tile([128, 4, ns], F32R, name="o4T")
    nc.vector.tensor_copy(o4T, po4t)

    pcls = psum.tile([ns, 512], F32, name="pcls", tag="ps1", bufs=1)
    for j in range(4):
        _mm(nc, pcls[:, 0:16], o4T[:, j, :], W.wcls[:, j, :],
            start=(j == 0), stop=(j == 3 and not flags["cls_bias"]))
    if flags["cls_bias"]:
        _mm(nc, pcls[:, 0:16], ones, W.bcrow, start=False, stop=True)
    outsb = fc.tile([ns, 16], F32, name="outsb")
    nc.vector.tensor_copy(outsb, pcls[:, 0:16])
    nc.sync.dma_start(out=out_dram[:], in_=outsb)


def build_nc(wvals, flags, n_samples=S, debug=False):
    nc = bass.Bass()
    x_dram = nc.declare_dram_parameter("x", [n_samples, C, NTOK], F32R, isOutput=False)
    out_dram = nc.declare_dram_parameter("out", [n_samples, 16], F32, isOutput=True)
    taps = [] if debug else None

    with tile.TileContext(nc) as tc:
        with (
            tc.tile_pool(name="wts", bufs=1) as wts,
            tc.tile_pool(name="acts", bufs=2) as acts,
            tc.tile_pool(name="stats", bufs=3) as stats,
            tc.tile_pool(name="fc", bufs=1) as fc,
            tc.tile_pool(name="psum", bufs=1, space="PSUM") as psum,
        ):
            pools = {"acts": acts, "stats": stats, "psum": psum, "fc": fc}
            W = _load_weights(nc, wts, wvals)
            O2buf = fc.tile([128, 25, n_samples], F32R, name="O2buf")
            grp = {}
            for s in range(n_samples):
                if s % CGRP == 0:
                    grp["o1p"] = acts.tile([64, CGRP, 148], BF16, name="o1p_grp")
                _emit_sample(nc, pools, W, flags, x_dram, s, O2buf, grp, taps)
                if s % CGRP == CGRP - 1 or s == n_samples - 1:
                    g0 = (s // CGRP) * CGRP
                    _emit_conv2_group(nc, pools, W, O2buf, grp, g0, s - g0 + 1)
            _emit_fc(nc, pools, W, flags, out_dram, O2buf, n_samples)

    _split_waits(nc)
    return nc, taps


_BUILD_CACHE = {}


def kernel(**inputs):
    wvals, flags = _prep_weights(inputs)
    key = tuple(sorted(flags.items()))
    if key not in _BUILD_CACHE:
        _BUILD_CACHE[key] = build_nc(wvals, flags)
    nc, _ = _BUILD_CACHE[key]

    x = np.ascontiguousarray(np.asarray(inputs["x"], np.float32)).reshape(
        N_CORES, S, C, NTOK
    )
    in_maps = []
    for c in range(N_CORES):
        m = {"x": np.ascontiguousarray(x[c])}
        m.update(wvals)
        in_maps.append(m)
    last_err = None
    for _attempt in range(3):
        try:
            res = run_bass_kernel_spmd(nc, in_maps, core_ids=list(range(N_CORES)))
            break
        except Exception as e:  # transient device faults: retry
            last_err = e
    else:
        raise last_err
    out = np.concatenate([res.results[c]["out"] for c in range(N_CORES)], axis=0)
    return out.astype(np.float32)


# revision 16
# speedup vs baseline: 1.9298x; 1.1471x over previous
"""Trainium2 Bass kernel for nn_Discriminator_AddDim_ESSAAttn.

Network (per sample, C=128, 27x27 spatial, N=729 tokens):
  ESSA linear attention -> concat -> 1x1-conv FFN (+residual) ->
  3x3 conv/relu/pool x2 -> 3 FC layers -> [16] logits.
Batch 256 is sharded 32-per-core across 8 NeuronCores (pure data
parallel, weights replicated). Everything for one sample lives in SBUF;
weights are resident.

Layout strategy:
  - qkv computed token-major ([tokens, 384] psum tiles) so per-token
    stats (channel means/sums/norms) are per-partition scalars.
    Channel-mean subtraction is folded into the qkv weights host-side
    (centering is linear: w_q' = w_q - mean_j(w_q)).
  - q2-row-normalisation folded into per-partition tensor_scalar muls;
    k2 column (token-dim) L2 norm folded into a post-matmul per-channel
    scale using the gram matrix diag computed in the same PE pass.
  - kv/t2/attn/convs run channel-major; q2n is PE-transposed.
  - v (channel-major) and the ffn2 residual are accumulated directly in
    PSUM by extra matmuls (w_v / identity as stationary).
  - convs are 9 accumulating matmuls over shifted [C, H, W] views;
    conv2 batches 4 samples per matmul to keep the moving dim >= 256.
  - all matmuls use float32r (single-pass full-rate fp32 on the PE;
    plain fp32 is a 2-pass 4 cyc/col emulation).
"""
import sys

sys.path.insert(0, "/opt/trn_rl_repo")

import numpy as np

import concourse.bass as bass
import concourse.tile as tile
from concourse import mybir
from concourse.bass_utils import run_bass_kernel_spmd

F32 = mybir.dt.float32
F32R = mybir.dt.float32r
BF16 = mybir.dt.bfloat16
AF = mybir.ActivationFunctionType
ALU = mybir.AluOpType
AX = mybir.AxisListType

N_CORES = 8
B, C, P = 256, 128, 27
NTOK = P * P          # 729
S = B // N_CORES      # 32 samples per core
NT = 6                # token tiles: 5*128 + 89
TOK_SIZES = [128, 128, 128, 128, 128, 89]
CGRP = 4              # conv2 sample-group size


def _split_waits(nc, maxw=1):
    """walrus CoreV3 rejects instructions carrying >1 sem-wait; hoist
    extras onto preceding same-engine no-op carriers."""
    import bass_rust

    for bb in nc.m.functions[0].blocks:
        newlist = []
        for ins in bb.instructions:
            sw = ins.sync_info
            if sw and sw.on_wait and len(sw.on_wait) > maxw:
                waits = list(sw.on_wait)
                keep = waits[-maxw:]
                hoist = waits[:-maxw]
                for i in range(0, len(hoist), maxw):
                    chunk = hoist[i : i + maxw]
                    nop = bass_rust.InstNoOp(
                        name=f"{ins.name}_wsplit{i}", ins=[], outs=[]
                    )
                    nop.engine = ins.engine
                    nop.sync_info = mybir.SyncInfo(on_wait=list(chunk), on_update=[])
                    nc.register_instruction(nop, overwrite=True)
                    newlist.append(nop)
                ins.sync_info = mybir.SyncInfo(
                    on_wait=list(keep), on_update=list(sw.on_update)
                )
            newlist.append(ins)
        bb.instructions[:] = newlist


def _prep_weights(inputs):
    """Host-side weight massaging (all cheap numpy)."""
    f = lambda a: np.ascontiguousarray(np.asarray(a, np.float32))
    w_qkv = f(inputs["w_qkv"]).copy()          # [128, 384]
    b_qkv = f(inputs["b_qkv"]).copy()          # [384]
    # fold channel-mean subtraction of q and k into the weights/bias
    w_qkv[:, 0:128] -= w_qkv[:, 0:128].mean(axis=1, keepdims=True)
    w_qkv[:, 128:256] -= w_qkv[:, 128:256].mean(axis=1, keepdims=True)
    b_qkv[0:128] -= b_qkv[0:128].mean()
    b_qkv[128:256] -= b_qkv[128:256].mean()

    w_ffn1 = f(inputs["w_ffn1"]).reshape(64, 256)     # [out, in]
    w1x = np.ascontiguousarray(w_ffn1[:, 0:128].T)    # [128, 64]
    w1a = np.ascontiguousarray(w_ffn1[:, 128:256].T)  # [128, 64]
    w2t = np.ascontiguousarray(f(inputs["w_ffn2"]).reshape(128, 64).T)  # [64, 128]

    # conv taps -> [in_ch, 9, out_ch]
    wc1 = np.ascontiguousarray(
        f(inputs["w_c1"]).transpose(2, 3, 1, 0).reshape(9, 128, 64).transpose(1, 0, 2)
    )  # [128, 9, 64]
    wc2 = np.ascontiguousarray(
        f(inputs["w_c2"]).transpose(2, 3, 1, 0).reshape(9, 64, 128).transpose(1, 0, 2)
    )  # [64, 9, 128]

    w1r = np.ascontiguousarray(f(inputs["w_fc1"]).reshape(128, 25, 512))
    wf2 = np.ascontiguousarray(f(inputs["w_fc2"]).reshape(4, 128, 512).transpose(1, 0, 2))
    wcls = np.ascontiguousarray(f(inputs["w_cls"]).reshape(4, 128, 16).transpose(1, 0, 2))

    col = lambda a: np.ascontiguousarray(f(a).reshape(-1, 1))
    row = lambda a: np.ascontiguousarray(f(a).reshape(1, -1))
    w = {
        "wqkv": w_qkv,
        "bqkv_row": row(b_qkv),
        "wln": f(inputs["w_ln"]),
        "bln": col(inputs["b_ln"]),
        "w1x": w1x,
        "w1a": w1a,
        "b1": col(inputs["b_ffn1"]),
        "w2t": w2t,
        "b2": col(inputs["b_ffn2"]),
        "wc1": wc1,
        "bc1": col(inputs["b_c1"]),
        "wc2": wc2,
        "bc2": col(inputs["b_c2"]),
        "w1r": w1r,
        "b1row": row(inputs["b_fc1"]),
        "wf2": wf2,
        "b2row": row(inputs["b_fc2"]),
        "wcls": wcls,
        "bcrow": row(inputs["b_cls"]),
        "eye": np.eye(128, dtype=np.float32),
        "ones1": np.ones((1, S), dtype=np.float32),
    }
    flags = {
        "qkv_bias": bool(np.any(b_qkv)),
        "fc1_bias": bool(np.any(w["b1row"])),
        "fc2_bias": bool(np.any(w["b2row"])),
        "cls_bias": bool(np.any(w["bcrow"])),
    }
    return w, flags


class _W:
    pass


_F32_WEIGHTS = {"bln", "b1", "b2", "bc1", "bc2"}  # activation-bias operands
_BF16_WEIGHTS = {"wln", "w1a", "w2t", "wc1", "wc2"}  # bf16 matmul path


def _load_weights(nc, pool, wvals):
    """Declare dram params + DMA every weight into resident SBUF tiles.
    bf16 weights are cast during a gpsimd DMA (only engine that casts)."""
    W = _W()
    for name, arr in wvals.items():
        if name in _F32_WEIGHTS:
            dt = F32
        elif name in _BF16_WEIGHTS:
            dt = BF16
        else:
            dt = F32R
        dram = nc.declare_dram_parameter(
            name, list(arr.shape), F32 if dt == BF16 else dt, isOutput=False
        )
        t = pool.tile(list(arr.shape), dt, name=f"sb_{name}")
        if dt == BF16:
            nc.gpsimd.dma_start(out=t, in_=dram[:])
        else:
            nc.sync.dma_start(out=t, in_=dram[:])
        setattr(W, name, t)
    return W


def _win(ap, offset, dims):
    """Manual sub-AP of a tile: dims = [[stride, count], ...] free dims.
    Rows may overlap (fp32r moving operands need inner size %4==0, so
    windows are padded to 28/12 wide and garbage columns dropped later)."""
    return bass.AP(
        tensor=ap.tensor, offset=ap.offset + offset,
        ap=[list(ap.ap[0])] + [list(d) for d in dims],
    )


def _mm(nc, out, lhsT, rhs, start=True, stop=True):
    nc.tensor.matmul(out, lhsT, rhs, start=start, stop=stop)


def _tp(nc, out, in_, eye):
    nc.tensor.matmul(
        out.bitcast(in_.dtype), in_, eye.bitcast(in_.dtype), is_transpose=True
    )


def _s0(nc, pools, W, flags, x_dram, s, taps):
    """qkv + token stats; leaves q2n/kvsrc(v|k2a) ready."""
    acts, stats, psum = pools["acts"], pools["stats"], pools["psum"]
    st = {"s": s}

    x_s = acts.tile([C, 768], F32R, name="x_s", bufs=3)
    nc.sync.dma_start(out=x_s[:, 0:NTOK], in_=x_dram[s])
    st["x_s"] = x_s

    q2 = acts.tile([128, NT, 128], F32, name="q2")
    k2 = acts.tile([128, NT, 128], F32, name="k2")
    kvsrc = acts.tile([128, NT, 256], F32R, name="kvsrc")  # [v | k2a]
    for half in range(2):
        pq = psum.tile([128, 3, 512], F32, name="pq", tag="pqkv", bufs=1)
        for i in range(3):
            t = half * 3 + i
            nt = TOK_SIZES[t]
            _mm(nc, pq[0:nt, i, 0:384], x_s[:, 128 * t : 128 * t + nt], W.wqkv,
                start=True, stop=not flags["qkv_bias"])
            if flags["qkv_bias"]:
                _mm(nc, pq[0:nt, i, 0:384], W.ones1[0:1, 0:nt], W.bqkv_row,
                    start=False, stop=True)
        h3 = slice(3 * half, 3 * half + 3)
        nc.scalar.activation(q2[:, h3, :], pq[:, :, 0:128], AF.Square)
        nc.scalar.activation(k2[:, h3, :], pq[:, :, 128:256], AF.Square)
        nc.vector.tensor_copy(kvsrc[:, h3, 0:128], pq[:, :, 256:384])

    sq2 = stats.tile([128, NT], F32, name="sq2")
    sk2 = stats.tile([128, NT], F32, name="sk2")
    sq4 = stats.tile([128, NT], F32, name="sq4")
    q4s = acts.tile([128, 128], F32, name="q4s")
    nc.vector.reduce_sum(sq2, q2, axis=AX.X)
    nc.vector.reduce_sum(sk2, k2, axis=AX.X)
    for t in range(NT):
        nc.vector.scalar_tensor_tensor(
            out=q4s, in0=q2[:, t, :], scalar=1.0, in1=q2[:, t, :],
            op0=ALU.mult, op1=ALU.mult, accum_out=sq4[:, t : t + 1],
        )

    s1q = stats.tile([128, NT], F32, name="s1q")
    nc.vector.tensor_scalar_add(s1q, sq2, 1e-7)
    nc.vector.reciprocal(s1q, s1q)
    nq = stats.tile([128, NT], F32, name="nq")
    nc.scalar.activation(nq, sq4, AF.Sqrt)
    cq = stats.tile([128, NT], F32, name="cq")
    nc.vector.tensor_mul(cq, s1q, nq)
    nc.vector.tensor_scalar_max(cq, cq, 1e-12)
    nc.vector.reciprocal(cq, cq)
    nc.vector.tensor_mul(cq, cq, s1q)
    s1k = stats.tile([128, NT], F32, name="s1k")
    nc.vector.tensor_scalar_add(s1k, sk2, 1e-7)
    nc.vector.reciprocal(s1k, s1k)

    q2n = acts.tile([128, NT, 128], F32R, name="q2n")
    for t in range(NT):
        nc.vector.tensor_scalar_mul(q2n[:, t, :], q2[:, t, :], cq[:, t : t + 1])
    for t in range(NT):
        nc.vector.tensor_scalar_mul(
            kvsrc[:, t, 128:256], k2[:, t, :], s1k[:, t : t + 1]
        )
    st.update(q2n=q2n, kvsrc=kvsrc, q2=q2)
    return st


def _s1(nc, pools, W, st, taps):
    """kv+gram, transpose, t2+v, attn."""
    acts, stats, psum = pools["acts"], pools["stats"], pools["psum"]
    x_s, kvsrc, q2n = st["x_s"], st["kvsrc"], st["q2n"]

    pkv = psum.tile([128, 512], F32, name="pkv", tag="ps1", bufs=1)
    for t in range(NT):
        nt = TOK_SIZES[t]
        _mm(nc, pkv[:, 0:256], kvsrc[0:nt, t, 128:256], kvsrc[0:nt, t, :],
            start=(t == 0), stop=(t == NT - 1))
    tmpd = acts.tile([128, 128], F32, name="tmpd")
    s2 = stats.tile([128, 1], F32, name="s2")
    nc.vector.tensor_mul(tmpd, pkv[:, 128:256], W.eye)
    nc.vector.reduce_sum(s2, tmpd, axis=AX.X)
    invs = stats.tile([128, 1], F32, name="invs")
    nc.scalar.activation(invs, s2, AF.Sqrt, scale=float(NTOK))  # 27*sqrt(s2)
    nc.vector.tensor_scalar_max(invs, invs, 27e-12)
    nc.vector.reciprocal(invs, invs)
    kvsb = acts.tile([128, 128], F32R, name="kvsb")
    nc.scalar.mul(kvsb, pkv[:, 0:128], invs)

    pqt = psum.tile([128, 768], F32, name="pqt", tag="ps2", bufs=2)
    for t in range(NT):
        _tp(nc, pqt[:, 128 * t : 128 * (t + 1)], q2n[:, t, :], W.eye)
    q2nT = acts.tile([128, 768], F32R, name="q2nT")
    nc.vector.tensor_copy(q2nT, pqt)

    wv = W.wqkv[:, 256:384]
    pt2 = psum.tile([128, 768], F32, name="pt2", tag="ps2", bufs=2)
    _mm(nc, pt2[:, 0:512], kvsb, q2nT[:, 0:512], start=True, stop=False)
    _mm(nc, pt2[:, 0:512], wv, x_s[:, 0:512], start=False, stop=True)
    _mm(nc, pt2[:, 512:768], kvsb, q2nT[:, 512:768], start=True, stop=False)
    _mm(nc, pt2[:, 512:768], wv, x_s[:, 512:768], start=False, stop=True)
    vt = acts.tile([C, 732], BF16, name="vt")
    nc.scalar.copy(vt[:, 0:NTOK], pt2[:, 0:729])

    pat = psum.tile([128, 768], F32, name="pat", tag="ps2", bufs=2)
    _mm(nc, pat[:, 0:512], W.wln, vt[:, 0:512])
    _mm(nc, pat[:, 512:732], W.wln, vt[:, 512:732])
    attn = acts.tile([C, 732], BF16, name="attn", bufs=3)
    nc.scalar.activation(attn[:, 0:NTOK], pat[:, 0:729], AF.Identity, bias=W.bln)
    st.update(vt=vt, attn=attn, kvsb=kvsb, q2nT=q2nT)


def _s2(nc, pools, W, st, grp, taps):
    """ffn1, ffn2+residual, conv1+pool."""
    acts, psum = pools["acts"], pools["psum"]
    x_s, attn, s = st["x_s"], st["attn"], st["s"]

    ph = psum.tile([64, 768], F32, name="ph", tag="ps2", bufs=2)
    _mm(nc, ph[:, 0:512], W.w1x, x_s[:, 0:512], start=True, stop=False)
    _mm(nc, ph[:, 512:768], W.w1x, x_s[:, 512:768], start=True, stop=False)
    _mm(nc, ph[:, 0:512], W.w1a, attn[:, 0:512], start=False, stop=True)
    _mm(nc, ph[:, 512:732], W.w1a, attn[:, 512:732], start=False, stop=True)
    h0 = acts.tile([64, NTOK], F32, name="h0")
    nc.scalar.activation(h0, ph[:, 0:729], AF.Identity, bias=W.b1)
    h = acts.tile([64, 732], BF16, name="h")
    nc.vector.scalar_tensor_tensor(
        out=h[:, 0:NTOK], in0=h0, scalar=0.01, in1=h0, op0=ALU.mult, op1=ALU.max
    )

    pxen = psum.tile([128, 768], F32, name="pxen", tag="ps2", bufs=2)
    _mm(nc, pxen[:, 0:512], W.w2t, h[:, 0:512], start=True, stop=True)
    _mm(nc, pxen[:, 512:732], W.w2t, h[:, 512:732], start=True, stop=True)
    xen = acts.tile([C, 768], BF16, name="xen", bufs=3)
    # residual + bias folded into the extract (saves two identity matmuls)
    nc.vector.scalar_tensor_tensor(
        out=xen[:, 0:NTOK], in0=pxen[:, 0:729], scalar=W.b2, in1=x_s[:, 0:NTOK],
        op0=ALU.add, op1=ALU.add,
    )

    pc1a = psum.tile([64, 13, 28], F32, name="pc1a", tag="ps1", bufs=1)
    pc1b = psum.tile([64, 12, 28], F32, name="pc1b", tag="ps2", bufs=2)
    for ky in range(3):
        for kx in range(3):
            tap = ky * 3 + kx
            # 28-wide overlapping windows; 3 garbage cols/row dropped later
            _mm(nc, pc1a, W.wc1[:, tap, :],
                _win(xen, ky * 27 + kx, [[27, 13], [1, 28]]),
                start=(tap == 0), stop=(tap == 8))
            _mm(nc, pc1b, W.wc1[:, tap, :],
                _win(xen, (ky + 13) * 27 + kx, [[27, 12], [1, 28]]),
                start=(tap == 0), stop=(tap == 8))
    o1r = acts.tile([64, 625], BF16, name="o1r")
    o1rv = o1r.rearrange("p (h w) -> p h w", h=25)
    nc.scalar.activation(o1rv[:, 0:13, :], pc1a[:, :, 0:25], AF.Relu, bias=W.bc1)
    nc.scalar.activation(o1rv[:, 13:25, :], pc1b[:, :, 0:25], AF.Relu, bias=W.bc1)
    o1r3 = o1r.rearrange("p (h w) -> p h w", h=25)
    m1 = acts.tile([64, 144], BF16, name="m1")
    m1v = m1.rearrange("p (a b) -> p a b", a=12)
    m2 = acts.tile([64, 144], BF16, name="m2")
    m2v = m2.rearrange("p (a b) -> p a b", a=12)
    g = s % CGRP
    o1pv = grp["o1p"][:, g, 0:144].rearrange("p (a b) -> p a b", a=12)
    nc.vector.tensor_max(m1v, o1r3[:, 0:24:2, 0:24:2], o1r3[:, 0:24:2, 1:25:2])
    nc.vector.tensor_max(m2v, o1r3[:, 1:25:2, 0:24:2], o1r3[:, 1:25:2, 1:25:2])
    nc.vector.tensor_max(o1pv, m1v, m2v)

    if taps is not None and s == 0:
        for nm, t in (
            ("q2n", st["q2n"]), ("kvsb", st["kvsb"]), ("attn", attn),
            ("xen", xen), ("o1p", grp["o1p"][:, 0, :]), ("vt", st["vt"]),
            ("q2", st["q2"]),
        ):
            d = nc.declare_dram_parameter(f"tap_{nm}", list(t.shape), t.dtype, isOutput=True)
            nc.sync.dma_start(out=d[:], in_=t)
            taps.append(f"tap_{nm}")


def _emit_conv2_group(nc, pools, W, O2buf, grp, g0, gn):
    """conv2+pool for a group of gn samples (moving dim = gn*100)."""
    acts, psum = pools["acts"], pools["psum"]
    pc2 = psum.tile([128, CGRP, 10, 12], F32, name="pc2", tag="ps1", bufs=1)
    for ky in range(3):
        for kx in range(3):
            tap = ky * 3 + kx
            # 12-wide overlapping windows (inner %4); 2 garbage cols/row
            _mm(nc, pc2[:, 0:gn], W.wc2[:, tap, :],
                _win(grp["o1p"], ky * 12 + kx, [[148, gn], [12, 10], [1, 12]]),
                start=(tap == 0), stop=(tap == 8))
    o2r = acts.tile([128, CGRP, 100], BF16, name="o2r")
    o2rv = o2r.rearrange("p g (h w) -> p g h w", h=10)
    nc.scalar.activation(o2rv[:, 0:gn], pc2[:, 0:gn, :, 0:10], AF.Relu, bias=W.bc2)
    o2r4 = o2rv
    n1 = acts.tile([128, CGRP, 25], F32, name="n1")
    n1v = n1.rearrange("p g (a b) -> p g a b", a=5)
    n2 = acts.tile([128, CGRP, 25], F32, name="n2")
    n2v = n2.rearrange("p g (a b) -> p g a b", a=5)
    nc.vector.tensor_max(
        n1v[:, 0:gn], o2r4[:, 0:gn, 0:10:2, 0:10:2], o2r4[:, 0:gn, 0:10:2, 1:10:2]
    )
    nc.vector.tensor_max(
        n2v[:, 0:gn], o2r4[:, 0:gn, 1:10:2, 0:10:2], o2r4[:, 0:gn, 1:10:2, 1:10:2]
    )
    outv = (
        O2buf[:, :, g0 : g0 + gn]
        .rearrange("p a g -> p g a")
        .rearrange("p g (a b) -> p g a b", a=5)
    )
    nc.vector.tensor_max(outv, n1v[:, 0:gn], n2v[:, 0:gn])


def _emit_fc(nc, pools, W, flags, out_dram, O2buf, ns):
    psum, fc = pools["psum"], pools["fc"]
    ones = W.ones1[0:1, 0:ns]

    po3 = psum.tile([ns, 512], F32, name="po3", tag="ps1", bufs=1)
    for p in range(25):
        _mm(nc, po3, O2buf[:, p, :], W.w1r[:, p, :],
            start=(p == 0), stop=(p == 24 and not flags["fc1_bias"]))
    if flags["fc1_bias"]:
        _mm(nc, po3, ones, W.b1row, start=False, stop=True)
    o3r = fc.tile([ns, 512], F32R, name="o3r")
    nc.scalar.activation(o3r, po3, AF.Relu)

    po3t = psum.tile([128, 4, ns], F32, name="po3t", tag="ps1", bufs=1)
    for j in range(4):
        _tp(nc, po3t[:, j, :], o3r[:, 128 * j : 128 * (j + 1)], W.eye[0:ns, 0:ns])
    o3T = fc.tile([128, 4, ns], F32R, name="o3T")
    nc.vector.tensor_copy(o3T, po3t)

    po4 = psum.tile([ns, 512], F32, name="po4", tag="ps1", bufs=1)
    for j in range(4):
        _mm(nc, po4, o3T[:, j, :], W.wf2[:, j, :],
            start=(j == 0), stop=(j == 3 and not flags["fc2_bias"]))
    if flags["fc2_bias"]:
        _mm(nc, po4, ones, W.b2row, start=False, stop=True)
    o4r = fc.tile([ns, 512], F32R, name="o4r")
    nc.scalar.activation(o4r, po4, AF.Relu)

    po4t = psum.tile([128, 4, ns], F32, name="po4t", tag="ps1", bufs=1)
    for j in range(4):
        _tp(nc, po4t[:, j, :], o4r[:, 128 * j : 128 * (j + 1)], W.eye[0:ns, 0:ns])
    o4T = fc.tile([128, 4, ns], F32R, name="o4T")
    nc.vector.tensor_copy(o4T, po4t)

    pcls = psum.tile([ns, 512], F32, name="pcls", tag="ps1", bufs=1)
    for j in range(4):
        _mm(nc, pcls[:, 0:16], o4T[:, j, :], W.wcls[:, j, :],
            start=(j == 0), stop=(j == 3 and not flags["cls_bias"]))
    if flags["cls_bias"]:
        _mm(nc, pcls[:, 0:16], ones, W.bcrow, start=False, stop=True)
    outsb = fc.tile([ns, 16], F32, name="outsb")
    nc.vector.tensor_copy(outsb, pcls[:, 0:16])
    nc.sync.dma_start(out=out_dram[:], in_=outsb)


def build_nc(wvals, flags, n_samples=S, debug=False):
    nc = bass.Bass()
    x_dram = nc.declare_dram_parameter("x", [n_samples, C, NTOK], F32R, isOutput=False)
    out_dram = nc.declare_dram_parameter("out", [n_samples, 16], F32, isOutput=True)
    taps = [] if debug else None

    with tile.TileContext(nc) as tc:
        with (
            tc.tile_pool(name="wts", bufs=1) as wts,
            tc.tile_pool(name="acts", bufs=2) as acts,
            tc.tile_pool(name="stats", bufs=3) as stats,
            tc.tile_pool(name="fc", bufs=1) as fc,
            tc.tile_pool(name="psum", bufs=1, space="PSUM") as psum,
        ):
            pools = {"acts": acts, "stats": stats, "psum": psum, "fc": fc}
            W = _load_weights(nc, wts, wvals)
            O2buf = fc.tile([128, 25, n_samples], F32R, name="O2buf")
            grp = {}
            # 2-sample software pipeline: stage k of sample b is emitted
            # right after stage k of sample a, so the PE stream has another
            # sample's matmuls to chew on while extracts run.
            for p0 in range(0, n_samples, 2):
                pair = [p0] + ([p0 + 1] if p0 + 1 < n_samples else [])
                sts = []
                for s in pair:
                    if s % CGRP == 0:
                        grp["o1p"] = acts.tile([64, CGRP, 148], BF16, name="o1p_grp")
                    sts.append(_s0(nc, pools, W, flags, x_dram, s, taps))
                for st in sts:
                    _s1(nc, pools, W, st, taps)
                for st in sts:
                    _s2(nc, pools, W, st, grp, taps)
                s_last = pair[-1]
                if s_last % CGRP == CGRP - 1 or s_last == n_samples - 1:
                    g0 = (s_last // CGRP) * CGRP
                    _emit_conv2_group(nc, pools, W, O2buf, grp, g0, s_last - g0 + 1)
            _emit_fc(nc, pools, W, flags, out_dram, O2buf, n_samples)

    _split_waits(nc)
    return nc, taps


_BUILD_CACHE = {}


def kernel(**inputs):
    wvals, flags = _prep_weights(inputs)
    key = tuple(sorted(flags.items()))
    if key not in _BUILD_CACHE:
        _BUILD_CACHE[key] = build_nc(wvals, flags)
    nc, _ = _BUILD_CACHE[key]

    x = np.ascontiguousarray(np.asarray(inputs["x"], np.float32)).reshape(
        N_CORES, S, C, NTOK
    )
    in_maps = []
    for c in range(N_CORES):
        m = {"x": np.ascontiguousarray(x[c])}
        m.update(wvals)
        in_maps.append(m)
    last_err = None
    for _attempt in range(3):
        try:
            res = run_bass_kernel_spmd(nc, in_maps, core_ids=list(range(N_CORES)))
            break
        except Exception as e:  # transient device faults: retry
            last_err = e
    else:
        raise last_err
    out = np.concatenate([res.results[c]["out"] for c in range(N_CORES)], axis=0)
    return out.astype(np.float32)
